# revision 1
# baseline (speedup 1.0000x reference)
"""Trainium2 Bass kernel for a Gaussian-splat rendering loss.

Full inputs -> scalar loss. Sharding: 8 cores = 2 batches x 4 row-bands.
Each core renders a 38-row window (32 owned rows + 3-row halo each side)
of one batch image against all 1024 depth-sorted gaussians, computes its
partial loss sums on-device, and the host combines 8 partial vectors.

Device algorithm (per core):
  - per-gaussian preprocessing (projection, EWA 2D covariance, colors)
  - splat power computed as a PE matmul: power[pix,n] = Phi[pix,:] @ Psi[:,n]
    where Phi are pixel monomials [gx^2, gx*gy, gy^2, gx, gy, 1] (centered)
  - alpha/transmittance compositing via a multiplicative scan over sorted
    gaussians; weighted color/depth reduction via Abel summation
    (sum_n w[n]*col[n] = col[0] + sum_n c[n]*(col[n+1]-col[n]))
  - separable 7x7 gaussian SSIM on the rendered window
  - L1 / SSIM / depth / opacity-entropy partial sums -> [6] outputs
"""

import os
import numpy as np

B, N, H, W = 2, 1024, 128, 128
R = 38          # window rows per core (32 owned + 3 halo each side)
OWN = 32
NCORES = 8
C0 = 0.28209479177387814
C1 = 0.01 ** 2
C2 = 0.03 ** 2
EXP_N10 = float(np.exp(np.float32(-10.0)))  # exp(-10) in f32

NPIX_RGB = float(B * 3 * H * W)
NPIX_D = float(B * 1 * H * W)
NGAUSS = float(B * N)


def _ssim_g7():
    coords = np.arange(7, dtype=np.float32) - 3
    g = np.exp(-coords ** 2 / (2 * np.float32(1.5) ** 2))
    g = g / g.sum()
    return g.astype(np.float32)

G7 = _ssim_g7()


# --------------------------------------------------------------------------
# host-side sharding
# --------------------------------------------------------------------------

def shard_inputs(gaussians, intrinsics, target_rgb, target_depth):
    gaussians = np.ascontiguousarray(gaussians, dtype=np.float32)
    intrinsics = np.ascontiguousarray(intrinsics, dtype=np.float32)
    target_rgb = np.ascontiguousarray(target_rgb, dtype=np.float32)
    target_depth = np.ascontiguousarray(target_depth, dtype=np.float32)

    z = np.maximum(gaussians[:, :, 2], 1e-4)
    order = np.argsort(z, axis=1, kind="stable")
    gs = np.take_along_axis(gaussians, order[:, :, None], axis=1)  # [B,N,38]

    gx = np.arange(W, dtype=np.float32) - 64.0
    in_maps = []
    for c in range(NCORES):
        b, q = divmod(c, 4)
        row0 = q * OWN
        wr = np.arange(row0 - 3, row0 + OWN + 3)
        valid = (wr >= 0) & (wr < H)
        wrc = np.clip(wr, 0, H - 1)

        gyv = np.where(valid, wr.astype(np.float32) - 64.0, 0.0).astype(np.float32)
        phi = np.zeros((R, 6, W), np.float32)
        phi[:, 0, :] = gx * gx
        phi[:, 1, :] = gyv[:, None] * gx
        phi[:, 2, :] = (gyv * gyv)[:, None]
        phi[:, 3, :] = gx
        phi[:, 4, :] = gyv[:, None]
        phi[:, 5, :] = 1.0

        targ4 = np.zeros((4, R, W), np.float32)
        targ4[0:3, valid, :] = target_rgb[b][:, wrc[valid], :]
        targ4[3, valid, :] = target_depth[b, 0, wrc[valid], :]

        rowmask = np.zeros((128, 1), np.float32)
        rowmask[: 3 * R, 0] = np.tile(valid.astype(np.float32), 3)

        opac_slice = gs[b, q * 256:(q + 1) * 256, 10].astype(np.float32)
        oe = np.ascontiguousarray(opac_slice.reshape(2, 128).T)  # [128,2]

        in_maps.append({
            "g38": np.ascontiguousarray(gs[b]),
            "intr9": np.ascontiguousarray(intrinsics[b].reshape(1, 9)),
            "phi": phi,
            "targ4": targ4,
            "rowmask": rowmask,
            "oe": oe,
        })
    return in_maps


def combine(partials_list):
    S = np.zeros(6, np.float64)
    for p in partials_list:
        S += p.astype(np.float64)
    l1_rgb = (S[0] + S[1] + S[2]) / NPIX_RGB
    l1_depth = S[3] / NPIX_D
    ssim = S[4] / NPIX_RGB
    ent = -S[5] / NGAUSS
    loss = 0.8 * l1_rgb + 0.2 * (1.0 - ssim) + 0.5 * l1_depth + 0.01 * ent
    return np.float32(loss)


# --------------------------------------------------------------------------
# numpy mirror of the device program (for algorithm validation)
# --------------------------------------------------------------------------

def _prep_gaussians_np(g, intr):
    """Per-gaussian preprocessing, mirrors the device ops in f32."""
    f = np.float32
    g = g.astype(f)
    x, y, z3 = g[:, 0], g[:, 1], g[:, 2]
    s0, s1, s2 = g[:, 3], g[:, 4], g[:, 5]
    qw, qx, qy, qz = g[:, 6], g[:, 7], g[:, 8], g[:, 9]
    opac = g[:, 10]
    intr = intr.reshape(9)
    fx, cx, fy, cy = intr[0], intr[2], intr[4], intr[5]

    zcl = np.maximum(z3, f(1e-4))
    rz = f(1.0) / zcl
    px = (x * rz) * fx + cx
    py = (y * rz) * fy + cy
    zc6 = np.maximum(z3, f(1e-6))
    rzc = f(1.0) / zc6
    aJ = rzc * fx
    cJ = rzc * fy
    rzsq = rzc * rzc
    bJ = (x * rzsq) * (-fx)
    dJ = (y * rzsq) * (-fy)

    xx, yy, zz = qx * qx, qy * qy, qz * qz
    xy, xz, yz = qx * qy, qx * qz, qy * qz
    wx, wy, wz = qw * qx, qw * qy, qw * qz
    r00 = (yy + zz) * f(-2) + f(1)
    r01 = (xy - wz) * f(2)
    r02 = (xz + wy) * f(2)
    r10 = (xy + wz) * f(2)
    r11 = (xx + zz) * f(-2) + f(1)
    r12 = (yz - wx) * f(2)
    r20 = (xz - wy) * f(2)
    r21 = (yz + wx) * f(2)
    r22 = (xx + yy) * f(-2) + f(1)
    ss0, ss1, ss2 = s0 * s0, s1 * s1, s2 * s2
    t00, t01, t02 = r00 * ss0, r01 * ss1, r02 * ss2
    t10, t11, t12 = r10 * ss0, r11 * ss1, r12 * ss2
    t20, t21, t22 = r20 * ss0, r21 * ss1, r22 * ss2
    Ca = (r00 * t00 + r01 * t01) + r02 * t02
    Cb = (r00 * t10 + r01 * t11) + r02 * t12
    Cc = (r00 * t20 + r01 * t21) + r02 * t22
    Cd = (r10 * t10 + r11 * t11) + r12 * t12
    Ce = (r10 * t20 + r11 * t21) + r12 * t22
    Cf = (r20 * t20 + r21 * t21) + r22 * t22

    a2, ab, b2 = aJ * aJ, aJ * bJ, bJ * bJ
    c2, cd, d2 = cJ * cJ, cJ * dJ, dJ * dJ
    c00 = (a2 * Ca + b2 * Cf) + (ab * Cc) * f(2) + f(0.3)
    c11 = (c2 * Cd + d2 * Cf) + (cd * Ce) * f(2) + f(0.3)
    ac, ad, bc, bd = aJ * cJ, aJ * dJ, bJ * cJ, bJ * dJ
    c01 = (ac * Cb + ad * Cc) + (bc * Ce + bd * Cf)
    det = np.maximum(c00 * c11 - c01 * c01, f(1e-8))
    rdet = f(1.0) / det
    i00 = c11 * rdet
    i11 = c00 * rdet
    ni01 = c01 * rdet  # = -inv01

    pxc = px - f(64)
    pyc = py - f(64)
    psi = np.zeros((6, N), np.float32)
    psi[0] = i00 * f(-0.5)
    psi[1] = ni01
    psi[2] = i11 * f(-0.5)
    psi[3] = i00 * pxc - ni01 * pyc
    psi[4] = i11 * pyc - ni01 * pxc
    psi[5] = (pxc * psi[3] + pyc * psi[4]) * f(-0.5)

    col = np.clip(g[:, 11:14] * f(C0) + f(0.5), 0.0, 1.0).astype(f)  # [N,3]
    colz = np.concatenate([col, zcl[:, None]], axis=1)  # [N,4]
    dcolz = np.empty_like(colz)
    dcolz[:-1] = colz[1:] - colz[:-1]
    dcolz[-1] = -colz[-1]
    return psi, colz, dcolz, opac


def _conv7_np(x, axis):
    """SAME zero-padded 7-tap conv along given axis, f32, mirrors device order."""
    out = np.zeros_like(x)
    n = x.shape[axis]
    xm = np.moveaxis(x, axis, 0)
    om = np.moveaxis(out, axis, 0)
    om[:] = xm * G7[3]
    for k in [0, 1, 2, 4, 5, 6]:
        lo = max(0, 3 - k)
        hi = n + min(0, 3 - k)
        om[lo:hi] += xm[lo + k - 3: hi + k - 3] * G7[k]
    return out


def mirror_core(m):
    """Numpy mirror of one core's device program. Returns partials [6]."""
    f = np.float32
    psi, colz, dcolz, opac = _prep_gaussians_np(m["g38"], m["intr9"])
    phi = m["phi"]  # [R, 6, W]

    # render
    rend = np.zeros((4, R, W), np.float32)
    negop = -opac
    for r in range(R):
        power = (phi[r].T.astype(f) @ psi.astype(f)).astype(f)  # [W, N]
        e = np.exp(power).astype(f)
        mneg = np.maximum(e, f(EXP_N10)) * negop[None, :]
        oma = np.maximum(mneg + f(1.0), f(0.01)).astype(f)
        c = np.cumprod(oma, axis=1, dtype=f)  # [W, N]
        acc = (c @ dcolz.astype(f)).astype(f)  # [W, 4]
        rend[0:3, r, :] = np.maximum(acc[:, 0:3] + colz[0, 0:3], f(0.0)).T
        rend[3, r, :] = acc[:, 3] + colz[0, 3]
    rend[0:3] = np.minimum(rend[0:3], f(1.0))

    # l1 losses (owned rows only)
    omask = np.zeros((4, R, W), np.float32)
    omask[:, 3:3 + OWN, :] = 1.0
    l1d = np.abs(rend - m["targ4"]).astype(f)
    lacc = (l1d * omask).reshape(4, -1).sum(axis=1, dtype=f)

    # ssim on the window
    rowmask = m["rowmask"][: 3 * R, 0].reshape(3, R)
    img1 = rend[0:3] * rowmask[:, :, None]
    img2 = m["targ4"][0:3]
    i11 = img1 * img1
    i22 = img2 * img2
    i12 = img1 * img2
    outs = []
    for xin in (img1, img2, i11, i22, i12):
        rc = _conv7_np(xin.astype(f), axis=2)     # along W
        hc = _conv7_np(rc.astype(f), axis=1)      # along rows (full window)
        outs.append(hc[:, 3:3 + OWN, :].astype(f))
    mu1, mu2, M11, M22, M12 = outs
    A = mu1 * mu2
    num = (A * f(2) + f(C1)) * ((M12 - A) * f(2) + f(C2))
    Cq = mu1 * mu1
    Dq = mu2 * mu2
    den = ((Cq + f(C1)) + Dq) * (((M11 - Cq) + f(C2)) + (M22 - Dq))
    smap = (num / den).astype(f)
    ssum = smap.sum(dtype=f)

    # entropy partial
    o = np.clip(m["oe"], f(1e-6), f(1.0 - 1e-6)).astype(f)
    ent = (o * np.log(o) + (f(1.0) - o) * np.log(f(1.0) - o)).sum(dtype=f)

    return np.array([lacc[0], lacc[1], lacc[2], lacc[3], ssum, ent], np.float32)


def kernel_numpy(**inputs):
    """Full numpy mirror (no device) - for validation."""
    in_maps = shard_inputs(**inputs)
    partials = [mirror_core(m) for m in in_maps]
    return combine(partials)


# --------------------------------------------------------------------------
# device program
# --------------------------------------------------------------------------

F32 = None  # set on first build (mybir import deferred so numpy path stays light)
_PROG_CACHE = {}


def build_program(debug_rend=False):
    import concourse.bass as bass
    import concourse.bacc as bacc
    import concourse.tile as tile
    import concourse.mybir as mybir
    from concourse.masks import make_identity

    F32 = mybir.dt.float32
    OP = mybir.AluOpType
    ACT = mybir.ActivationFunctionType

    nc = bacc.Bacc("TRN2", target_bir_lowering=False, debug=False,
                   num_devices=NCORES)
    g38 = nc.dram_tensor("g38", [N, 38], F32, kind="ExternalInput").ap()
    intr9 = nc.dram_tensor("intr9", [1, 9], F32, kind="ExternalInput").ap()
    phi_in = nc.dram_tensor("phi", [R, 6, W], F32, kind="ExternalInput").ap()
    targ4_in = nc.dram_tensor("targ4", [4, R, W], F32, kind="ExternalInput").ap()
    rowmask_in = nc.dram_tensor("rowmask", [128, 1], F32, kind="ExternalInput").ap()
    oe_in = nc.dram_tensor("oe", [128, 2], F32, kind="ExternalInput").ap()
    partials = nc.dram_tensor("partials", [6], F32, kind="ExternalOutput").ap()
    if debug_rend:
        dbg_rend = nc.dram_tensor("dbg_rend", [4, R, W], F32, kind="ExternalOutput").ap()

    V = nc.vector
    S = nc.scalar
    T = nc.tensor
    G = nc.gpsimd

    with tile.TileContext(nc) as tc:
        with (
            tc.tile_pool(name="const", bufs=1) as cp,
            tc.tile_pool(name="prep", bufs=1) as pp,
            tc.tile_pool(name="loop", bufs=2) as lp,
            tc.tile_pool(name="ppow", bufs=2, space="PSUM") as ppow,
            tc.tile_pool(name="pct", bufs=1, space="PSUM") as pct,
            tc.tile_pool(name="pmisc", bufs=1, space="PSUM") as pmisc,
            tc.tile_pool(name="dram", bufs=1, space="DRAM") as dp,
        ):
            # ---------------- constants / loads ----------------
            idt = cp.tile([128, 128], F32, tag="identity", name="identity")
            make_identity(nc, idt[:])
            ones_col = cp.tile([128, 1], F32, tag="ones_col", name="ones_col")
            G.memset(ones_col[:], 1.0)

            gall = cp.tile([128, 8, 38], F32, tag="gall", name="gall")
            nc.sync.dma_start(gall[:], g38.rearrange("(f p) c -> p f c", p=128))

            intr_sb = cp.tile([1, 9], F32, tag="intr_sb", name="intr_sb")
            nc.sync.dma_start(intr_sb[:], intr9[:])
            ones_row = cp.tile([1, 128], F32, tag="ones_row", name="ones_row")
            G.memset(ones_row[:], 1.0)
            intrb = cp.tile([128, 9], F32, tag="intrb", name="intrb")
            bps = pmisc.tile([128, 128], F32, tag="tp", name="bps")
            T.matmul(bps[:, 0:9], ones_row[:], intr_sb[:], start=True, stop=True)
            V.tensor_copy(intrb[:], bps[:, 0:9])
            fx = intrb[:, 0:1]
            cxs = intrb[:, 2:3]
            fy = intrb[:, 4:5]
            cys = intrb[:, 5:6]

            phi_all = cp.tile([6, R, W], F32, tag="phi_all", name="phi_all")
            nc.sync.dma_start(phi_all[:], phi_in.rearrange("r k w -> k r w"))

            targ4_sb = cp.tile([4, R, W], F32, tag="targ4_sb", name="targ4_sb")
            nc.sync.dma_start(targ4_sb[:], targ4_in[:])
            targc = cp.tile([128, W], F32, tag="targc", name="targc")
            G.memset(targc[:], 0.0)
            nc.sync.dma_start(targc[0:114, :], targ4_in[0:3].rearrange("c r w -> (c r) w"))
            rowm = cp.tile([128, 1], F32, tag="rowm", name="rowm")
            nc.sync.dma_start(rowm[:], rowmask_in[:])
            oe = cp.tile([128, 2], F32, tag="oe", name="oe")
            nc.sync.dma_start(oe[:], oe_in[:])

            omask = cp.tile([4, R, W], F32, tag="omask", name="omask")
            G.memset(omask[:], 0.0)
            G.memset(omask[:, 3:3 + OWN, :], 1.0)

            # ---------------- per-gaussian preprocessing ----------------
            _tc = [0]

            def t8(tag=None):
                if tag is None:
                    _tc[0] += 1
                    tag = f"tmp{_tc[0]}"
                return pp.tile([128, 8], F32, tag=tag, name=tag)

            def mul(a, b):
                o = t8()
                V.tensor_mul(o[:], a[:], b[:])
                return o

            def add(a, b):
                o = t8()
                V.tensor_add(o[:], a[:], b[:])
                return o

            def sub(a, b):
                o = t8()
                V.tensor_sub(o[:], a[:], b[:])
                return o

            def ts(a, s1, op0, s2=None, op1=OP.bypass):
                o = t8()
                V.tensor_scalar(o[:], a[:], s1, s2, op0, op1)
                return o

            def stt(a, s, b, op0, op1):
                o = t8()
                V.scalar_tensor_tensor(o[:], a[:], s, b[:], op0, op1)
                return o

            def recip(a):
                o = t8()
                V.reciprocal(o[:], a[:])
                return o

            gx_ = gall[:, :, 0]
            gy_ = gall[:, :, 1]
            gz_ = gall[:, :, 2]

            negfx = cp.tile([128, 1], F32, tag="negfx", name="negfx")
            V.tensor_scalar(negfx[:], fx, -1.0, None, OP.mult, OP.bypass)
            negfy = cp.tile([128, 1], F32, tag="negfy", name="negfy")
            V.tensor_scalar(negfy[:], fy, -1.0, None, OP.mult, OP.bypass)

            class W_:  # wrap raw AP slices so helpers can call [:]
                def __init__(self, ap):
                    self.ap = ap

                def __getitem__(self, k):
                    return self.ap

            xw, yw, zw = W_(gx_), W_(gy_), W_(gz_)

            zcl = ts(zw, 1e-4, OP.max)
            rz = recip(zcl)
            t0 = mul(xw, rz)
            px = ts(t0, fx, OP.mult, cxs, OP.add)
            t1 = mul(yw, rz)
            py = ts(t1, fy, OP.mult, cys, OP.add)
            zc6 = ts(zw, 1e-6, OP.max)
            rzc = recip(zc6)
            aJ = ts(rzc, fx, OP.mult)
            cJ = ts(rzc, fy, OP.mult)
            rzsq = mul(rzc, rzc)
            t2 = mul(xw, rzsq)
            bJ = ts(t2, negfx[:, 0:1], OP.mult)
            t3 = mul(yw, rzsq)
            dJ = ts(t3, negfy[:, 0:1], OP.mult)

            qw = W_(gall[:, :, 6])
            qx = W_(gall[:, :, 7])
            qy = W_(gall[:, :, 8])
            qz = W_(gall[:, :, 9])
            xx, yy, zz = mul(qx, qx), mul(qy, qy), mul(qz, qz)
            xy, xz, yz = mul(qx, qy), mul(qx, qz), mul(qy, qz)
            wx, wy, wz = mul(qw, qx), mul(qw, qy), mul(qw, qz)

            def rentry(u, neg=False):
                if neg:
                    return ts(u, -2.0, OP.mult, 1.0, OP.add)
                return ts(u, 2.0, OP.mult)

            r00 = rentry(add(yy, zz), neg=True)
            r01 = rentry(sub(xy, wz))
            r02 = rentry(add(xz, wy))
            r10 = rentry(add(xy, wz))
            r11 = rentry(add(xx, zz), neg=True)
            r12 = rentry(sub(yz, wx))
            r20 = rentry(sub(xz, wy))
            r21 = rentry(add(yz, wx))
            r22 = rentry(add(xx, yy), neg=True)

            ss0, ss1, ss2 = mul(W_(gall[:, :, 3]), W_(gall[:, :, 3])), \
                mul(W_(gall[:, :, 4]), W_(gall[:, :, 4])), \
                mul(W_(gall[:, :, 5]), W_(gall[:, :, 5]))
            t00, t01, t02 = mul(r00, ss0), mul(r01, ss1), mul(r02, ss2)
            t10, t11, t12 = mul(r10, ss0), mul(r11, ss1), mul(r12, ss2)
            t20, t21, t22 = mul(r20, ss0), mul(r21, ss1), mul(r22, ss2)

            def dot3(a1, b1, a2, b2, a3, b3):
                u = add(mul(a1, b1), mul(a2, b2))
                return add(u, mul(a3, b3))

            Ca = dot3(r00, t00, r01, t01, r02, t02)
            Cb = dot3(r00, t10, r01, t11, r02, t12)
            Cc = dot3(r00, t20, r01, t21, r02, t22)
            Cd = dot3(r10, t10, r11, t11, r12, t12)
            Ce = dot3(r10, t20, r11, t21, r12, t22)
            Cf = dot3(r20, t20, r21, t21, r22, t22)

            a2, ab, b2 = mul(aJ, aJ), mul(aJ, bJ), mul(bJ, bJ)
            c2, cd, d2 = mul(cJ, cJ), mul(cJ, dJ), mul(dJ, dJ)

            u = add(mul(a2, Ca), mul(b2, Cf))
            c00 = stt(mul(ab, Cc), 2.0, u, OP.mult, OP.add)
            c00 = ts(c00, 0.3, OP.add)
            u = add(mul(c2, Cd), mul(d2, Cf))
            c11 = stt(mul(cd, Ce), 2.0, u, OP.mult, OP.add)
            c11 = ts(c11, 0.3, OP.add)
            ac, ad, bc, bd = mul(aJ, cJ), mul(aJ, dJ), mul(bJ, cJ), mul(bJ, dJ)
            u = add(mul(ac, Cb), mul(ad, Cc))
            v = add(mul(bc, Ce), mul(bd, Cf))
            c01 = add(u, v)

            det = sub(mul(c00, c11), mul(c01, c01))
            det = ts(det, 1e-8, OP.max)
            rdet = recip(det)
            i00 = mul(c11, rdet)
            i11 = mul(c00, rdet)
            ni01 = mul(c01, rdet)

            pxc = ts(px, -64.0, OP.add)
            pyc = ts(py, -64.0, OP.add)
            psi0 = ts(i00, -0.5, OP.mult)
            psi2 = ts(i11, -0.5, OP.mult)
            psi3 = sub(mul(i00, pxc), mul(ni01, pyc))
            psi4 = sub(mul(i11, pyc), mul(ni01, pxc))
            u = add(mul(pxc, psi3), mul(pyc, psi4))
            psi5 = ts(u, -0.5, OP.mult)

            def colch(k):
                c = ts(W_(gall[:, :, 11 + k]), C0, OP.mult, 0.5, OP.add)
                return ts(c, 0.0, OP.max, 1.0, OP.min)

            col0, col1, col2 = colch(0), colch(1), colch(2)
            negop = ts(W_(gall[:, :, 10]), -1.0, OP.mult)

            # ---------------- DMA shuffles via DRAM scratch ----------------
            shuf = dp.tile([16, N], F32, tag="shuf", name="shuf")

            def out_row(k, tl):
                nc.sync.dma_start(shuf[k].rearrange("(f p) -> p f", p=128), tl[:])

            for k, tl in enumerate([psi0, ni01, psi2, psi3, psi4, psi5,
                                    col0, col1, col2, zcl]):
                out_row(k, tl)
            out_row(14, negop)

            Psi = cp.tile([6, N], F32, tag="Psi", name="Psi")
            nc.sync.dma_start(Psi[:], shuf[0:6, :])
            colzr = cp.tile([4, N], F32, tag="colzr", name="colzr")
            nc.sync.dma_start(colzr[:], shuf[6:10, :])
            dcol = cp.tile([4, N], F32, tag="dcol", name="dcol")
            V.tensor_sub(dcol[:, 0:N - 1], colzr[:, 1:N], colzr[:, 0:N - 1])
            V.tensor_scalar(dcol[:, N - 1:N], colzr[:, N - 1:N], -1.0, None,
                            OP.mult, OP.bypass)
            nc.sync.dma_start(shuf[10:14, :], dcol[:])
            dcz = cp.tile([128, 8, 4], F32, tag="dcz", name="dcz")
            for q in range(4):
                nc.sync.dma_start(dcz[:, :, q],
                                  shuf[10 + q].rearrange("(f p) -> p f", p=128))
            c0z0 = cp.tile([4, 1], F32, tag="c0z0", name="c0z0")
            nc.sync.dma_start(c0z0[:], shuf[6:10, 0:1])
            nrow = cp.tile([1, N], F32, tag="nrow", name="nrow")
            nc.sync.dma_start(nrow[:], shuf[14:15, :])
            negopb = cp.tile([128, N], F32, tag="negopb", name="negopb")
            nps = ppow.tile([128, N], F32, tag="pow", name="nps")
            T.matmul(nps[:, 0:512], ones_row[:], nrow[:, 0:512], start=True, stop=True)
            T.matmul(nps[:, 512:1024], ones_row[:], nrow[:, 512:1024], start=True, stop=True)
            V.tensor_copy(negopb[:], nps[:])

            # ---------------- render loop ----------------
            rend_all = cp.tile([4, R, W], F32, tag="rend_all", name="rend_all")
            NK = N // 128
            for r in range(R):
                pw = ppow.tile([128, N], F32, tag="pow", name="pow")
                T.matmul(pw[:, 0:512], phi_all[:, r, :], Psi[:, 0:512],
                         start=True, stop=True)
                T.matmul(pw[:, 512:1024], phi_all[:, r, :], Psi[:, 512:1024],
                         start=True, stop=True)
                er = lp.tile([128, N], F32, tag="eraw", name="eraw")
                S.activation(er[:], pw[:], ACT.Exp, bias=0.0, scale=1.0)
                mn = lp.tile([128, N], F32, tag="mneg", name="mneg")
                V.scalar_tensor_tensor(mn[:], er[:], EXP_N10, negopb[:],
                                       OP.max, OP.mult)
                om = lp.tile([128, N], F32, tag="oma", name="oma")
                V.tensor_scalar(om[:], mn[:], 1.0, 0.01, OP.add, OP.max)
                ct = lp.tile([128, N], F32, tag="ctile", name="ctile")
                V.tensor_tensor_scan(ct[:], om[:], om[:], 1.0, OP.mult, OP.bypass)
                cps = pct.tile([128, NK, 128], F32, tag="ct", name="ct")
                for k in range(NK):
                    T.transpose(cps[:, k, :], ct[:, k * 128:(k + 1) * 128], idt[:])
                csb = lp.tile([128, NK, 128], F32, tag="ctsb", name="ctsb")
                V.tensor_copy(csb[:], cps[:])
                acc = pmisc.tile([4, 128], F32, tag="acc", name="acc")
                for k in range(NK):
                    T.matmul(acc[:], dcz[:, k, :], csb[:, k, :],
                             start=(k == 0), stop=(k == NK - 1))
                V.tensor_scalar(rend_all[:, r, :], acc[:, :], c0z0[:, :],
                                0.0, OP.add, OP.max)

            V.tensor_scalar(rend_all[0:3], rend_all[0:3], 1.0, None,
                            OP.min, OP.bypass)

            if debug_rend:
                nc.sync.dma_start(dbg_rend[:], rend_all[:])

            # ---------------- L1 losses ----------------
            l1d = cp.tile([4, R, W], F32, tag="l1d", name="l1d")
            V.tensor_sub(l1d[:], rend_all[:], targ4_sb[:])
            S.activation(l1d[:], l1d[:], ACT.Abs, bias=0.0, scale=1.0)
            lacc = cp.tile([4, 1], F32, tag="lacc", name="lacc")
            V.tensor_mul(l1d[:], l1d[:], omask[:])
            V.tensor_reduce(lacc[:], l1d[:], axis=mybir.AxisListType.XY, op=OP.add)

            # ---------------- SSIM ----------------
            img1 = cp.tile([128, W], F32, tag="img1", name="img1")
            G.memset(img1[:], 0.0)
            for ch in range(3):
                nc.sync.dma_start(img1[ch * R:(ch + 1) * R, :],
                                  rend_all[ch:ch + 1, :, :])
            V.tensor_scalar(img1[:], img1[:], rowm[:], None,
                            OP.mult, OP.bypass)
            i11t = cp.tile([128, W], F32, tag="i11t", name="i11t")
            V.tensor_mul(i11t[:], img1[:], img1[:])
            i22t = cp.tile([128, W], F32, tag="i22t", name="i22t")
            V.tensor_mul(i22t[:], targc[:], targc[:])
            i12t = cp.tile([128, W], F32, tag="i12t", name="i12t")
            V.tensor_mul(i12t[:], img1[:], targc[:])

            g7 = [float(v) for v in G7]
            convs = []
            for j, xin in enumerate([img1, targc, i11t, i22t, i12t]):
                rc = cp.tile([128, W], F32, tag=f"rc{j}", name=f"rc{j}")
                V.tensor_scalar(rc[:], xin[:], g7[3], None, OP.mult, OP.bypass)
                for k in [0, 1, 2, 4, 5, 6]:
                    lo = max(0, 3 - k)
                    hi = W + min(0, 3 - k)
                    V.scalar_tensor_tensor(rc[:, lo:hi], xin[:, lo + k - 3:hi + k - 3],
                                           g7[k], rc[:, lo:hi], OP.mult, OP.add)
                tp = pmisc.tile([128, 128], F32, tag="tp", name="tp")
                T.transpose(tp[:], rc[:], idt[:])
                rcT = cp.tile([128, 128], F32, tag=f"rcT{j}", name=f"rcT{j}")
                V.tensor_copy(rcT[:], tp[:])
                rcv = rcT[:, 0:114].rearrange("p (c r) -> p c r", c=3)
                mu = cp.tile([128, 3, OWN], F32, tag=f"mu{j}", name=f"mu{j}")
                V.tensor_scalar(mu[:], rcv[:, :, 3:3 + OWN], g7[3], None,
                                OP.mult, OP.bypass)
                for k in [0, 1, 2, 4, 5, 6]:
                    V.scalar_tensor_tensor(mu[:], rcv[:, :, k:k + OWN], g7[k],
                                           mu[:], OP.mult, OP.add)
                convs.append(mu)
            mu1, mu2, M11, M22, M12 = convs

            def big(tag):
                return cp.tile([128, 3, OWN], F32, tag=tag, name=tag)

            A = big("ssA")
            V.tensor_mul(A[:], mu1[:], mu2[:])
            num1 = big("ssnum1")
            V.tensor_scalar(num1[:], A[:], 2.0, C1, OP.mult, OP.add)
            Bv = big("ssB")
            V.tensor_sub(Bv[:], M12[:], A[:])
            num2 = big("ssnum2")
            V.tensor_scalar(num2[:], Bv[:], 2.0, C2, OP.mult, OP.add)
            num = big("ssnum")
            V.tensor_mul(num[:], num1[:], num2[:])
            Cq = big("ssC")
            V.tensor_mul(Cq[:], mu1[:], mu1[:])
            Dq = big("ssD")
            V.tensor_mul(Dq[:], mu2[:], mu2[:])
            den1 = big("ssden1")
            V.scalar_tensor_tensor(den1[:], Cq[:], C1, Dq[:], OP.add, OP.add)
            Ev = big("ssE")
            V.tensor_sub(Ev[:], M11[:], Cq[:])
            Fv = big("ssF")
            V.tensor_sub(Fv[:], M22[:], Dq[:])
            den2 = big("ssden2")
            V.scalar_tensor_tensor(den2[:], Ev[:], C2, Fv[:], OP.add, OP.add)
            den = big("ssden")
            V.tensor_mul(den[:], den1[:], den2[:])
            rden = big("ssrden")
            V.reciprocal(rden[:], den[:])
            smap = big("ssmap")
            V.tensor_mul(smap[:], num[:], rden[:])
            ssum = cp.tile([128, 1], F32, tag="ssum", name="ssum")
            V.tensor_reduce(ssum[:], smap[:], axis=mybir.AxisListType.XY, op=OP.add)
            sp = pmisc.tile([1, 1], F32, tag="tp", name="tp")
            T.matmul(sp[:], ssum[:], ones_col[:], start=True, stop=True)

            # ---------------- entropy ----------------
            ocl = cp.tile([128, 2], F32, tag="ocl", name="ocl")
            V.tensor_scalar(ocl[:], oe[:], 1e-6, 1.0 - 1e-6, OP.max, OP.min)
            lno = cp.tile([128, 2], F32, tag="lno", name="lno")
            S.activation(lno[:], ocl[:], ACT.Ln, bias=0.0, scale=1.0)
            e1 = cp.tile([128, 2], F32, tag="ent_e1", name="ent_e1")
            V.tensor_mul(e1[:], ocl[:], lno[:])
            omm = cp.tile([128, 2], F32, tag="ent_om", name="ent_om")
            V.tensor_scalar(omm[:], ocl[:], -1.0, 1.0, OP.mult, OP.add)
            lnm = cp.tile([128, 2], F32, tag="ent_lnm", name="ent_lnm")
            S.activation(lnm[:], omm[:], ACT.Ln, bias=0.0, scale=1.0)
            e2 = cp.tile([128, 2], F32, tag="ent_e2", name="ent_e2")
            V.tensor_mul(e2[:], omm[:], lnm[:])
            entt = cp.tile([128, 2], F32, tag="ent_t", name="ent_t")
            V.tensor_add(entt[:], e1[:], e2[:])
            esum = cp.tile([128, 1], F32, tag="esum", name="esum")
            V.tensor_reduce(esum[:], entt[:], axis=mybir.AxisListType.X, op=OP.add)
            ep = pmisc.tile([1, 1], F32, tag="tp", name="tp")
            T.matmul(ep[:], esum[:], ones_col[:], start=True, stop=True)

            # ---------------- outputs ----------------
            outsb = cp.tile([1, 2], F32, tag="outsb", name="outsb")
            V.tensor_copy(outsb[:, 0:1], sp[:])
            V.tensor_copy(outsb[:, 1:2], ep[:])
            nc.sync.dma_start(partials[0:4], lacc[:, 0])
            nc.sync.dma_start(partials[4:6], outsb[0, :])

    nc.compile()
    return nc


def _get_program(debug_rend=False):
    key = ("prog", debug_rend)
    if key not in _PROG_CACHE:
        _PROG_CACHE[key] = build_program(debug_rend)
    return _PROG_CACHE[key]


def run_device(in_maps, mode="hw", debug_rend=False):
    nc = _get_program(debug_rend)
    if mode == "sim":
        from concourse.bass_interp import MultiCoreSim
        sim = MultiCoreSim(nc, num_cores=len(in_maps))
        for i, m in enumerate(in_maps):
            for k, v in m.items():
                sim.cores[i].tensor(k)[:] = v
        sim.simulate(check_with_hw=False)
        names = ["partials"] + (["dbg_rend"] if debug_rend else [])
        return [{n: np.array(sim.cores[i].tensor(n)) for n in names}
                for i in range(len(in_maps))]
    from concourse.bass_utils import run_bass_kernel_spmd
    res = run_bass_kernel_spmd(nc, in_maps, list(range(len(in_maps))))
    return res.results


def kernel(**inputs):
    in_maps = shard_inputs(**inputs)
    mode = os.environ.get("GK_MODE", "hw")
    results = run_device(in_maps, mode=mode)
    return combine([r["partials"] for r in results])


if __name__ == "__main__":
    import jax
    with jax.default_device(jax.devices("cpu")[0]):
        import reference
        inputs = {k: np.asarray(v) for k, v in reference.setup_inputs().items()}
        expected = float(reference.reference(**inputs))
    got = float(kernel_numpy(**inputs))
    rel = abs(got - expected) / max(abs(expected), 1e-12)
    print(f"expected {expected:.8f}  mirror {got:.8f}  rel {rel:.3e}")



# revision 5
# speedup vs baseline: 43.3278x; 43.3278x over previous
"""Trainium2 Bass kernel for a Gaussian-splat rendering loss.

Full inputs -> scalar loss. Sharding: 8 cores = 2 batches x 4 row-bands.
Each core renders a 38-row window (32 owned rows + 3-row halo each side)
of one batch image against all 1024 depth-sorted gaussians, computes its
partial loss sums on-device, and the host combines 8 partial vectors.

Per-gaussian preprocessing (projection, EWA 2D covariance, colors) runs
on the host (tiny: O(N) numpy) so each core only receives the compact
derived tensors; the pixel-monomial matrix Phi is reconstructed on device
from a small basis (rank-3 factorization) instead of being shipped.

Device algorithm (per core):
  - splat power computed as a PE matmul: power[pix,n] = Phi[pix,:] @ Psi[:,n]
    where Phi are pixel monomials [gx^2, gx*gy, gy^2, gx, gy, 1] (centered)
  - alpha/transmittance compositing via a multiplicative scan over sorted
    gaussians; weighted color/depth reduction via Abel summation
    (sum_n w[n]*col[n] = col[0] + sum_n c[n]*(col[n+1]-col[n]))
  - separable 7x7 gaussian SSIM on the rendered window
  - L1 / SSIM / depth / opacity-entropy partial sums -> [6] outputs
"""

import os
import numpy as np

B, N, H, W = 2, 1024, 128, 128
R = 38          # window rows per core (32 owned + 3 halo each side)
OWN = 32
NCORES = 8
C0 = 0.28209479177387814
C1 = 0.01 ** 2
C2 = 0.03 ** 2
EXP_N10 = float(np.exp(np.float32(-10.0)))  # exp(-10) in f32

NPIX_RGB = float(B * 3 * H * W)
NPIX_D = float(B * 1 * H * W)
NGAUSS = float(B * N)


def _ssim_g7():
    coords = np.arange(7, dtype=np.float32) - 3
    g = np.exp(-coords ** 2 / (2 * np.float32(1.5) ** 2))
    g = g / g.sum()
    return g.astype(np.float32)

G7 = _ssim_g7()


# --------------------------------------------------------------------------
# host-side per-gaussian preprocessing (numpy f32, O(N) - not the hot path)
# --------------------------------------------------------------------------

def _prep_gaussians_np(g, intr):
    """Projection + EWA 2D covariance + colors, all in f32."""
    f = np.float32
    g = g.astype(f)
    x, y, z3 = g[:, 0], g[:, 1], g[:, 2]
    s0, s1, s2 = g[:, 3], g[:, 4], g[:, 5]
    qw, qx, qy, qz = g[:, 6], g[:, 7], g[:, 8], g[:, 9]
    opac = g[:, 10]
    intr = intr.reshape(9)
    fx, cx, fy, cy = intr[0], intr[2], intr[4], intr[5]

    zcl = np.maximum(z3, f(1e-4))
    rz = f(1.0) / zcl
    px = (x * rz) * fx + cx
    py = (y * rz) * fy + cy
    zc6 = np.maximum(z3, f(1e-6))
    rzc = f(1.0) / zc6
    aJ = rzc * fx
    cJ = rzc * fy
    rzsq = rzc * rzc
    bJ = (x * rzsq) * (-fx)
    dJ = (y * rzsq) * (-fy)

    xx, yy, zz = qx * qx, qy * qy, qz * qz
    xy, xz, yz = qx * qy, qx * qz, qy * qz
    wx, wy, wz = qw * qx, qw * qy, qw * qz
    r00 = (yy + zz) * f(-2) + f(1)
    r01 = (xy - wz) * f(2)
    r02 = (xz + wy) * f(2)
    r10 = (xy + wz) * f(2)
    r11 = (xx + zz) * f(-2) + f(1)
    r12 = (yz - wx) * f(2)
    r20 = (xz - wy) * f(2)
    r21 = (yz + wx) * f(2)
    r22 = (xx + yy) * f(-2) + f(1)
    ss0, ss1, ss2 = s0 * s0, s1 * s1, s2 * s2
    t00, t01, t02 = r00 * ss0, r01 * ss1, r02 * ss2
    t10, t11, t12 = r10 * ss0, r11 * ss1, r12 * ss2
    t20, t21, t22 = r20 * ss0, r21 * ss1, r22 * ss2
    Ca = (r00 * t00 + r01 * t01) + r02 * t02
    Cb = (r00 * t10 + r01 * t11) + r02 * t12
    Cc = (r00 * t20 + r01 * t21) + r02 * t22
    Cd = (r10 * t10 + r11 * t11) + r12 * t12
    Ce = (r10 * t20 + r11 * t21) + r12 * t22
    Cf = (r20 * t20 + r21 * t21) + r22 * t22

    a2, ab, b2 = aJ * aJ, aJ * bJ, bJ * bJ
    c2, cd, d2 = cJ * cJ, cJ * dJ, dJ * dJ
    c00 = (a2 * Ca + b2 * Cf) + (ab * Cc) * f(2) + f(0.3)
    c11 = (c2 * Cd + d2 * Cf) + (cd * Ce) * f(2) + f(0.3)
    ac, ad, bc, bd = aJ * cJ, aJ * dJ, bJ * cJ, bJ * dJ
    c01 = (ac * Cb + ad * Cc) + (bc * Ce + bd * Cf)
    det = np.maximum(c00 * c11 - c01 * c01, f(1e-8))
    rdet = f(1.0) / det
    i00 = c11 * rdet
    i11 = c00 * rdet
    ni01 = c01 * rdet  # = -inv01

    pxc = px - f(64)
    pyc = py - f(64)
    psi = np.zeros((6, N), np.float32)
    psi[0] = i00 * f(-0.5)
    psi[1] = ni01
    psi[2] = i11 * f(-0.5)
    psi[3] = i00 * pxc - ni01 * pyc
    psi[4] = i11 * pyc - ni01 * pxc
    psi[5] = (pxc * psi[3] + pyc * psi[4]) * f(-0.5)

    col = np.clip(g[:, 11:14] * f(C0) + f(0.5), 0.0, 1.0).astype(f)  # [N,3]
    colz = np.concatenate([col, zcl[:, None]], axis=1)  # [N,4]
    dcolz = np.empty_like(colz)
    dcolz[:-1] = colz[1:] - colz[:-1]
    dcolz[-1] = -colz[-1]
    return psi, colz, dcolz, opac


# --------------------------------------------------------------------------
# host-side sharding
# --------------------------------------------------------------------------

def shard_inputs(gaussians, intrinsics, target_rgb, target_depth):
    gaussians = np.ascontiguousarray(gaussians, dtype=np.float32)
    intrinsics = np.ascontiguousarray(intrinsics, dtype=np.float32)
    target_rgb = np.ascontiguousarray(target_rgb, dtype=np.float32)
    target_depth = np.ascontiguousarray(target_depth, dtype=np.float32)

    z = np.maximum(gaussians[:, :, 2], 1e-4)
    order = np.argsort(z, axis=1, kind="stable")
    gs = np.take_along_axis(gaussians, order[:, :, None], axis=1)  # [B,N,38]

    # per-batch derived tensors
    per_b = []
    for b in range(B):
        psi, colz, dcolz, opac = _prep_gaussians_np(gs[b], intrinsics[b])
        dczT = np.ascontiguousarray(
            dcolz.reshape(8, 128, 4).transpose(1, 0, 2))  # [128(p),8(f),4]
        c0z0 = np.ascontiguousarray(colz[0].reshape(4, 1))
        negop = np.ascontiguousarray((-opac).reshape(1, N))
        per_b.append((psi, dczT, c0z0, negop, opac))

    gx = np.arange(W, dtype=np.float32) - 64.0
    basis = np.stack([gx * gx, gx, np.ones_like(gx)])  # [3, W]

    in_maps = []
    for c in range(NCORES):
        b, q = divmod(c, 4)
        psi, dczT, c0z0, negop, opac = per_b[b]
        row0 = q * OWN
        wr = np.arange(row0 - 3, row0 + OWN + 3)
        valid = (wr >= 0) & (wr < H)
        wrc = np.clip(wr, 0, H - 1)
        gyv = np.where(valid, wr.astype(np.float32) - 64.0, 0.0).astype(np.float32)

        # rank-3 factorization of Phi: phi[k,r,w] = sum_c basis[c,w]*CT[c,6r+k]
        CT = np.zeros((3, R * 6), np.float32)
        for r in range(R):
            gy = gyv[r]
            CT[0, 6 * r + 0] = 1.0              # gx^2
            CT[1, 6 * r + 1] = gy               # gy*gx
            CT[2, 6 * r + 2] = gy * gy          # gy^2
            CT[1, 6 * r + 3] = 1.0              # gx
            CT[2, 6 * r + 4] = gy               # gy
            CT[2, 6 * r + 5] = 1.0              # 1
        ctb = np.ascontiguousarray(np.concatenate([CT, basis], axis=1))  # [3, 228+128]

        targ4 = np.zeros((4, R, W), np.float32)
        targ4[0:3, valid, :] = target_rgb[b][:, wrc[valid], :]
        targ4[3, valid, :] = target_depth[b, 0, wrc[valid], :]

        rowmask = np.zeros((128, 1), np.float32)
        rowmask[: 3 * R, 0] = np.tile(valid.astype(np.float32), 3)

        opac_slice = opac[q * 256:(q + 1) * 256]
        oe = np.ascontiguousarray(opac_slice.reshape(2, 128).T)  # [128,2]

        in_maps.append({
            "Psi": psi,
            "dczT": dczT,
            "c0z0": c0z0,
            "negop": negop,
            "ctb": ctb,
            "targ4": targ4,
            "rowmask": rowmask,
            "oe": oe,
        })
    return in_maps


def combine(partials_list):
    S = np.zeros(6, np.float64)
    for p in partials_list:
        S += p.astype(np.float64)
    l1_rgb = (S[0] + S[1] + S[2]) / NPIX_RGB
    l1_depth = S[3] / NPIX_D
    ssim = S[4] / NPIX_RGB
    ent = -S[5] / NGAUSS
    loss = 0.8 * l1_rgb + 0.2 * (1.0 - ssim) + 0.5 * l1_depth + 0.01 * ent
    return np.float32(loss)


# --------------------------------------------------------------------------
# numpy mirror of the device program (for algorithm validation)
# --------------------------------------------------------------------------

def _conv7_np(x, axis):
    """SAME zero-padded 7-tap conv along given axis, f32, mirrors device order."""
    out = np.zeros_like(x)
    n = x.shape[axis]
    xm = np.moveaxis(x, axis, 0)
    om = np.moveaxis(out, axis, 0)
    om[:] = xm * G7[3]
    for k in [0, 1, 2, 4, 5, 6]:
        lo = max(0, 3 - k)
        hi = n + min(0, 3 - k)
        om[lo:hi] += xm[lo + k - 3: hi + k - 3] * G7[k]
    return out


def mirror_core(m):
    """Numpy mirror of one core's device program. Returns partials [6]."""
    f = np.float32
    psi = m["Psi"]
    negop = m["negop"][0]
    dcolz = m["dczT"].transpose(1, 0, 2).reshape(N, 4)
    c0z0 = m["c0z0"][:, 0]
    CT = m["ctb"][:, :R * 6]
    basis = m["ctb"][:, R * 6:]

    # render
    rend = np.zeros((4, R, W), np.float32)
    for r in range(R):
        phi_r = (CT[:, 6 * r:6 * r + 6].T @ basis).astype(f)    # [6, W]
        power = (phi_r.T.astype(f) @ psi.astype(f)).astype(f)   # [W, N]
        e = np.exp(power).astype(f)
        mneg = np.maximum(e, f(EXP_N10)) * negop[None, :]
        oma = np.maximum(mneg + f(1.0), f(0.01)).astype(f)
        c = np.cumprod(oma, axis=1, dtype=f)  # [W, N]
        acc = (c @ dcolz.astype(f)).astype(f)  # [W, 4]
        rend[0:3, r, :] = np.maximum(acc[:, 0:3] + c0z0[0:3], f(0.0)).T
        rend[3, r, :] = acc[:, 3] + c0z0[3]
    rend[0:3] = np.minimum(rend[0:3], f(1.0))

    # l1 losses (owned rows only)
    omask = np.zeros((4, R, W), np.float32)
    omask[:, 3:3 + OWN, :] = 1.0
    l1d = np.abs(rend - m["targ4"]).astype(f)
    lacc = (l1d * omask).reshape(4, -1).sum(axis=1, dtype=f)

    # ssim on the window
    rowmask = m["rowmask"][: 3 * R, 0].reshape(3, R)
    img1 = rend[0:3] * rowmask[:, :, None]
    img2 = m["targ4"][0:3]
    i11 = img1 * img1
    i22 = img2 * img2
    i12 = img1 * img2
    outs = []
    for xin in (img1, img2, i11, i22, i12):
        rc = _conv7_np(xin.astype(f), axis=2)     # along W
        hc = _conv7_np(rc.astype(f), axis=1)      # along rows (full window)
        outs.append(hc[:, 3:3 + OWN, :].astype(f))
    mu1, mu2, M11, M22, M12 = outs
    A = mu1 * mu2
    num = (A * f(2) + f(C1)) * ((M12 - A) * f(2) + f(C2))
    Cq = mu1 * mu1
    Dq = mu2 * mu2
    den = ((Cq + f(C1)) + Dq) * (((M11 - Cq) + f(C2)) + (M22 - Dq))
    smap = (num / den).astype(f)
    ssum = smap.sum(dtype=f)

    # entropy partial
    o = np.clip(m["oe"], f(1e-6), f(1.0 - 1e-6)).astype(f)
    ent = (o * np.log(o) + (f(1.0) - o) * np.log(f(1.0) - o)).sum(dtype=f)

    return np.array([lacc[0], lacc[1], lacc[2], lacc[3], ssum, ent], np.float32)


def kernel_numpy(**inputs):
    """Full numpy mirror (no device) - for validation."""
    in_maps = shard_inputs(**inputs)
    partials = [mirror_core(m) for m in in_maps]
    return combine(partials)


# --------------------------------------------------------------------------
# device program
# --------------------------------------------------------------------------

_PROG_CACHE = {}


def build_program(debug_rend=False):
    import concourse.bass as bass
    import concourse.bacc as bacc
    import concourse.tile as tile
    import concourse.mybir as mybir
    from concourse.masks import make_identity

    F32 = mybir.dt.float32
    OP = mybir.AluOpType
    ACT = mybir.ActivationFunctionType

    nc = bacc.Bacc("TRN2", target_bir_lowering=False, debug=False,
                   num_devices=NCORES)
    psi_in = nc.dram_tensor("Psi", [6, N], F32, kind="ExternalInput").ap()
    dczT_in = nc.dram_tensor("dczT", [128, 8, 4], F32, kind="ExternalInput").ap()
    c0z0_in = nc.dram_tensor("c0z0", [4, 1], F32, kind="ExternalInput").ap()
    negop_in = nc.dram_tensor("negop", [1, N], F32, kind="ExternalInput").ap()
    ctb_in = nc.dram_tensor("ctb", [3, R * 6 + W], F32, kind="ExternalInput").ap()
    targ4_in = nc.dram_tensor("targ4", [4, R, W], F32, kind="ExternalInput").ap()
    rowmask_in = nc.dram_tensor("rowmask", [128, 1], F32, kind="ExternalInput").ap()
    oe_in = nc.dram_tensor("oe", [128, 2], F32, kind="ExternalInput").ap()
    partials = nc.dram_tensor("partials", [6], F32, kind="ExternalOutput").ap()
    if debug_rend:
        dbg_rend = nc.dram_tensor("dbg_rend", [4, R, W], F32, kind="ExternalOutput").ap()

    V = nc.vector
    S = nc.scalar
    T = nc.tensor
    G = nc.gpsimd

    with tile.TileContext(nc) as tc:
        with (
            tc.tile_pool(name="const", bufs=1) as cp,
            tc.tile_pool(name="loop", bufs=2) as lp,
            tc.tile_pool(name="ppow", bufs=2, space="PSUM") as ppow,
            tc.tile_pool(name="pct", bufs=1, space="PSUM") as pct,
            tc.tile_pool(name="pmisc", bufs=1, space="PSUM") as pmisc,
            tc.tile_pool(name="dram", bufs=1, space="DRAM") as dp,
        ):
            # ---------------- constants / loads ----------------
            idt = cp.tile([128, 128], F32, tag="identity", name="identity")
            make_identity(nc, idt[:])
            ones_col = cp.tile([128, 1], F32, tag="ones_col", name="ones_col")
            G.memset(ones_col[:], 1.0)
            ones_row = cp.tile([1, 128], F32, tag="ones_row", name="ones_row")
            G.memset(ones_row[:], 1.0)

            Psi = cp.tile([6, N], F32, tag="Psi", name="Psi")
            nc.sync.dma_start(Psi[:], psi_in[:])
            dcz = cp.tile([128, 8, 4], F32, tag="dcz", name="dcz")
            nc.sync.dma_start(dcz[:], dczT_in[:])
            c0z0 = cp.tile([4, 1], F32, tag="c0z0", name="c0z0")
            nc.sync.dma_start(c0z0[:], c0z0_in[:])
            nrow = cp.tile([1, N], F32, tag="nrow", name="nrow")
            nc.sync.dma_start(nrow[:], negop_in[:])
            ctb = cp.tile([3, R * 6 + W], F32, tag="ctb", name="ctb")
            nc.sync.dma_start(ctb[:], ctb_in[:])

            targ4_sb = cp.tile([4, R, W], F32, tag="targ4_sb", name="targ4_sb")
            nc.sync.dma_start(targ4_sb[:], targ4_in[:])
            targc = cp.tile([128, W], F32, tag="targc", name="targc")
            G.memset(targc[:], 0.0)
            nc.sync.dma_start(targc[0:114, :], targ4_in[0:3].rearrange("c r w -> (c r) w"))
            rowm = cp.tile([128, 1], F32, tag="rowm", name="rowm")
            nc.sync.dma_start(rowm[:], rowmask_in[:])
            oe = cp.tile([128, 2], F32, tag="oe", name="oe")
            nc.sync.dma_start(oe[:], oe_in[:])

            omask = cp.tile([4, R, W], F32, tag="omask", name="omask")
            G.memset(omask[:], 0.0)
            G.memset(omask[:, 3:3 + OWN, :], 1.0)

            # ---------------- negop broadcast [128, N] ----------------
            negopb = cp.tile([128, N], F32, tag="negopb", name="negopb")
            nps = ppow.tile([128, N], F32, tag="pow", name="nps")
            T.matmul(nps[:, 0:512], ones_row[:], nrow[:, 0:512], start=True, stop=True)
            T.matmul(nps[:, 512:1024], ones_row[:], nrow[:, 512:1024], start=True, stop=True)
            V.tensor_copy(negopb[:], nps[:])

            # ---------------- Phi reconstruction ----------------
            # phiT[w, 6r+k] = sum_c basis[c,w] * CT[c,6r+k]; PE-transpose to
            # [(6r+k), w] chunks, bounce through DRAM to get the [6, R, W]
            # free-dim layout (matmul lhsT base partition must be 0/32/64).
            KR = R * 6  # 228
            bphi = pmisc.tile([128, KR], F32, tag="tp", name="bphi")
            T.matmul(bphi[:], ctb[:, KR:KR + W], ctb[:, 0:KR], start=True, stop=True)
            phiT_sb = cp.tile([128, KR], F32, tag="phiT_sb", name="phiT_sb")
            V.tensor_copy(phiT_sb[:], bphi[:])
            phiP = cp.tile([128, 2, 128], F32, tag="phiP", name="phiP")
            trA = pmisc.tile([128, 128], F32, tag="tp", name="trA")
            T.transpose(trA[0:126, :], phiT_sb[:, 0:126], idt[:])
            V.tensor_copy(phiP[0:126, 0, :], trA[0:126, :])
            trB = pmisc.tile([128, 128], F32, tag="tp", name="trB")
            T.transpose(trB[0:102, :], phiT_sb[:, 126:228], idt[:])
            V.tensor_copy(phiP[0:102, 1, :], trB[0:102, :])
            phi_scr = dp.tile([KR, W], F32, tag="phi_scr", name="phi_scr")
            nc.sync.dma_start(phi_scr[0:126, :], phiP[0:126, 0, :])
            nc.sync.dma_start(phi_scr[126:228, :], phiP[0:102, 1, :])
            phi_all = cp.tile([6, R, W], F32, tag="phi_all", name="phi_all")
            nc.sync.dma_start(phi_all[:],
                              phi_scr.rearrange("(r k) w -> k r w", k=6))

            # ---------------- render loop ----------------
            rend_all = cp.tile([4, R, W], F32, tag="rend_all", name="rend_all")
            NK = N // 128
            for r in range(R):
                pw = ppow.tile([128, N], F32, tag="pow", name="pow")
                T.matmul(pw[:, 0:512], phi_all[:, r, :], Psi[:, 0:512],
                         start=True, stop=True)
                T.matmul(pw[:, 512:1024], phi_all[:, r, :], Psi[:, 512:1024],
                         start=True, stop=True)
                er = lp.tile([128, N], F32, tag="eraw", name="eraw")
                S.activation(er[:], pw[:], ACT.Exp, bias=0.0, scale=1.0)
                mn = lp.tile([128, N], F32, tag="mneg", name="mneg")
                V.scalar_tensor_tensor(mn[:], er[:], EXP_N10, negopb[:],
                                       OP.max, OP.mult)
                om = lp.tile([128, N], F32, tag="oma", name="oma")
                V.tensor_scalar(om[:], mn[:], 1.0, 0.01, OP.add, OP.max)
                ct = lp.tile([128, N], F32, tag="ctile", name="ctile")
                V.tensor_tensor_scan(ct[:], om[:], om[:], 1.0, OP.mult, OP.bypass)
                cps = pct.tile([128, NK, 128], F32, tag="ct", name="ct")
                for k in range(NK):
                    T.transpose(cps[:, k, :], ct[:, k * 128:(k + 1) * 128], idt[:])
                csb = lp.tile([128, NK, 128], F32, tag="ctsb", name="ctsb")
                S.activation(csb[:], cps[:], ACT.Copy, bias=0.0, scale=1.0)
                acc = pmisc.tile([4, 128], F32, tag="acc", name="acc")
                for k in range(NK):
                    T.matmul(acc[:], dcz[:, k, :], csb[:, k, :],
                             start=(k == 0), stop=(k == NK - 1))
                V.tensor_scalar(rend_all[:, r, :], acc[:, :], c0z0[:, :],
                                0.0, OP.add, OP.max)

            V.tensor_scalar(rend_all[0:3], rend_all[0:3], 1.0, None,
                            OP.min, OP.bypass)

            if debug_rend:
                nc.sync.dma_start(dbg_rend[:], rend_all[:])

            # ---------------- L1 losses ----------------
            l1d = cp.tile([4, R, W], F32, tag="l1d", name="l1d")
            V.tensor_sub(l1d[:], rend_all[:], targ4_sb[:])
            S.activation(l1d[:], l1d[:], ACT.Abs, bias=0.0, scale=1.0)
            lacc = cp.tile([4, 1], F32, tag="lacc", name="lacc")
            V.tensor_mul(l1d[:], l1d[:], omask[:])
            V.tensor_reduce(lacc[:], l1d[:], axis=mybir.AxisListType.XY, op=OP.add)

            # ---------------- SSIM ----------------
            img1 = cp.tile([128, W], F32, tag="img1", name="img1")
            G.memset(img1[:], 0.0)
            for ch in range(3):
                nc.sync.dma_start(img1[ch * R:(ch + 1) * R, :],
                                  rend_all[ch:ch + 1, :, :])
            V.tensor_scalar(img1[:], img1[:], rowm[:], None,
                            OP.mult, OP.bypass)
            i11t = cp.tile([128, W], F32, tag="i11t", name="i11t")
            V.tensor_mul(i11t[:], img1[:], img1[:])
            i22t = cp.tile([128, W], F32, tag="i22t", name="i22t")
            V.tensor_mul(i22t[:], targc[:], targc[:])
            i12t = cp.tile([128, W], F32, tag="i12t", name="i12t")
            V.tensor_mul(i12t[:], img1[:], targc[:])

            g7 = [float(v) for v in G7]
            convs = []
            for j, xin in enumerate([img1, targc, i11t, i22t, i12t]):
                rc = cp.tile([128, W], F32, tag=f"rc{j}", name=f"rc{j}")
                V.tensor_scalar(rc[:], xin[:], g7[3], None, OP.mult, OP.bypass)
                for k in [0, 1, 2, 4, 5, 6]:
                    lo = max(0, 3 - k)
                    hi = W + min(0, 3 - k)
                    V.scalar_tensor_tensor(rc[:, lo:hi], xin[:, lo + k - 3:hi + k - 3],
                                           g7[k], rc[:, lo:hi], OP.mult, OP.add)
                tp = pmisc.tile([128, 128], F32, tag="tp", name="tp")
                T.transpose(tp[:], rc[:], idt[:])
                rcT = cp.tile([128, 128], F32, tag=f"rcT{j}", name=f"rcT{j}")
                V.tensor_copy(rcT[:], tp[:])
                rcv = rcT[:, 0:114].rearrange("p (c r) -> p c r", c=3)
                mu = cp.tile([128, 3, OWN], F32, tag=f"mu{j}", name=f"mu{j}")
                V.tensor_scalar(mu[:], rcv[:, :, 3:3 + OWN], g7[3], None,
                                OP.mult, OP.bypass)
                for k in [0, 1, 2, 4, 5, 6]:
                    V.scalar_tensor_tensor(mu[:], rcv[:, :, k:k + OWN], g7[k],
                                           mu[:], OP.mult, OP.add)
                convs.append(mu)
            mu1, mu2, M11, M22, M12 = convs

            def big(tag):
                return cp.tile([128, 3, OWN], F32, tag=tag, name=tag)

            A = big("ssA")
            V.tensor_mul(A[:], mu1[:], mu2[:])
            num1 = big("ssnum1")
            V.tensor_scalar(num1[:], A[:], 2.0, C1, OP.mult, OP.add)
            Bv = big("ssB")
            V.tensor_sub(Bv[:], M12[:], A[:])
            num2 = big("ssnum2")
            V.tensor_scalar(num2[:], Bv[:], 2.0, C2, OP.mult, OP.add)
            num = big("ssnum")
            V.tensor_mul(num[:], num1[:], num2[:])
            Cq = big("ssC")
            V.tensor_mul(Cq[:], mu1[:], mu1[:])
            Dq = big("ssD")
            V.tensor_mul(Dq[:], mu2[:], mu2[:])
            den1 = big("ssden1")
            V.scalar_tensor_tensor(den1[:], Cq[:], C1, Dq[:], OP.add, OP.add)
            Ev = big("ssE")
            V.tensor_sub(Ev[:], M11[:], Cq[:])
            Fv = big("ssF")
            V.tensor_sub(Fv[:], M22[:], Dq[:])
            den2 = big("ssden2")
            V.scalar_tensor_tensor(den2[:], Ev[:], C2, Fv[:], OP.add, OP.add)
            den = big("ssden")
            V.tensor_mul(den[:], den1[:], den2[:])
            rden = big("ssrden")
            V.reciprocal(rden[:], den[:])
            smap = big("ssmap")
            V.tensor_mul(smap[:], num[:], rden[:])
            ssum = cp.tile([128, 1], F32, tag="ssum", name="ssum")
            V.tensor_reduce(ssum[:], smap[:], axis=mybir.AxisListType.XY, op=OP.add)
            sp = pmisc.tile([1, 1], F32, tag="tp", name="tp2")
            T.matmul(sp[:], ssum[:], ones_col[:], start=True, stop=True)

            # ---------------- entropy ----------------
            ocl = cp.tile([128, 2], F32, tag="ocl", name="ocl")
            V.tensor_scalar(ocl[:], oe[:], 1e-6, 1.0 - 1e-6, OP.max, OP.min)
            lno = cp.tile([128, 2], F32, tag="lno", name="lno")
            S.activation(lno[:], ocl[:], ACT.Ln, bias=0.0, scale=1.0)
            e1 = cp.tile([128, 2], F32, tag="ent_e1", name="ent_e1")
            V.tensor_mul(e1[:], ocl[:], lno[:])
            omm = cp.tile([128, 2], F32, tag="ent_om", name="ent_om")
            V.tensor_scalar(omm[:], ocl[:], -1.0, 1.0, OP.mult, OP.add)
            lnm = cp.tile([128, 2], F32, tag="ent_lnm", name="ent_lnm")
            S.activation(lnm[:], omm[:], ACT.Ln, bias=0.0, scale=1.0)
            e2 = cp.tile([128, 2], F32, tag="ent_e2", name="ent_e2")
            V.tensor_mul(e2[:], omm[:], lnm[:])
            entt = cp.tile([128, 2], F32, tag="ent_t", name="ent_t")
            V.tensor_add(entt[:], e1[:], e2[:])
            esum = cp.tile([128, 1], F32, tag="esum", name="esum")
            V.tensor_reduce(esum[:], entt[:], axis=mybir.AxisListType.X, op=OP.add)
            ep = pmisc.tile([1, 1], F32, tag="tp", name="tp3")
            T.matmul(ep[:], esum[:], ones_col[:], start=True, stop=True)

            # ---------------- outputs ----------------
            outsb = cp.tile([1, 2], F32, tag="outsb", name="outsb")
            V.tensor_copy(outsb[:, 0:1], sp[:])
            V.tensor_copy(outsb[:, 1:2], ep[:])
            nc.sync.dma_start(partials[0:4], lacc[:, 0])
            nc.sync.dma_start(partials[4:6], outsb[0, :])

    nc.compile()
    return nc


def _get_program(debug_rend=False):
    key = ("prog", debug_rend)
    if key not in _PROG_CACHE:
        _PROG_CACHE[key] = build_program(debug_rend)
    return _PROG_CACHE[key]


def run_device(in_maps, mode="hw", debug_rend=False):
    nc = _get_program(debug_rend)
    if mode == "sim":
        from concourse.bass_interp import MultiCoreSim
        sim = MultiCoreSim(nc, num_cores=len(in_maps))
        for i, m in enumerate(in_maps):
            for k, v in m.items():
                sim.cores[i].tensor(k)[:] = v
        sim.simulate(check_with_hw=False)
        names = ["partials"] + (["dbg_rend"] if debug_rend else [])
        return [{n: np.array(sim.cores[i].tensor(n)) for n in names}
                for i in range(len(in_maps))]
    from concourse.bass_utils import run_bass_kernel_spmd
    res = run_bass_kernel_spmd(nc, in_maps, list(range(len(in_maps))))
    return res.results


def kernel(**inputs):
    in_maps = shard_inputs(**inputs)
    mode = os.environ.get("GK_MODE", "hw")
    results = run_device(in_maps, mode=mode)
    return combine([r["partials"] for r in results])


if __name__ == "__main__":
    import jax
    with jax.default_device(jax.devices("cpu")[0]):
        import reference
        inputs = {k: np.asarray(v) for k, v in reference.setup_inputs().items()}
        expected = float(reference.reference(**inputs))
    got = float(kernel_numpy(**inputs))
    rel = abs(got - expected) / max(abs(expected), 1e-12)
    print(f"expected {expected:.8f}  mirror {got:.8f}  rel {rel:.3e}")
